# revision 53
# baseline (speedup 1.0000x reference)
"""Trainium2 Bass kernel for banded (local-causal) multi-head self-attention.

Problem (hardcoded shapes): x [4, 2048, 1024], W_attn [1024, 3072],
b_attn [3072], W_proj [1024, 1024], b_proj [1024]; 16 heads, head dim 64,
local causal window 256.

Sharding over 8 NeuronCores: data-parallel over the 4 batches x
tensor-parallel over 2 head-groups (8 heads each). Each core computes a
partial projection output [2048, 1024]; the host sums the two head-group
partials per batch and adds b_proj.

Per-core device program (all loops fully unrolled under Tile):
  inputs: DMA queues drain serially per queue and share HBM between
          queues, so both queues front-load what phase A consumes
          first: a wv half each, then the x^T chunks on sync and the
          wqk/wp/mask weights on scalar.  ~40 warm matmuls on a memset
          tile (no DMA dependency) ramp the PE clock from ~1us so HAM
          never sees an idle start.
  phase A/B: all v tiles first (token-major [v_h|1] slots of stride 65,
          so PV emits the softmax denominator in its last column) --
          gated only on wv + x^T chunks -- then the q^T/k^T projection
          tiles (feature-major [feat, T]).
  phase C: per query block j: band-strip S^T matmuls for key block j+1
          (Exp on scalar; wedge masks merged into wide 4D-AP gpsimd
          ops), PV for two head pairs per PSUM tile (query-major
          [128, 4*65]; drain = one reciprocal + one stride-0-broadcast
          multiply into a packed a tile), a filler unit to hide the
          reciprocal->normalize vector chain, then the four [128,128]
          feature-block transposes: 2 on the sync XBAR + 2 on the PE
          (is_transpose matmuls into a bf16 PSUM tile) for early
          blocks; ALL 4 on the PE for blocks >= 10 whose projections
          follow at lag <= 1 (the PE path is ready ~2us sooner than the
          XBAR, which also carries the y output DMAs).  Dense filler
          (remaining q/k tiles, deferred v tiles, output projection at
          lag 2) keeps the PE issue-busy >98% so HAM holds 2.4 GHz; the
          last block splits its proj drains across scalar+vector and
          its y DMA into halves to shorten the final chain.

Matmul dtypes: bf16 everywhere (inputs pre-cast on host), fp32 PSUM accum.
"""

import numpy as np
import ml_dtypes

import concourse.bass as bass
import concourse.bacc as bacc
import concourse.mybir as mybir
import concourse.tile as tile
from concourse.bass import AP
from concourse.bass_utils import run_bass_kernel_spmd

B, T, C = 4, 2048, 1024
H, D, CTX = 16, 64, 256
HG = 8                 # heads per core
FG = HG * D            # 512 features per group
P = 128
NT = T // P            # 16 token blocks
KC = C // P            # 8 contraction tiles of C
W3 = 3 * P             # strip width 384
NWARM = 42             # PE clock-ramp warmup matmuls

BF16 = mybir.dt.bfloat16
F32 = mybir.dt.float32

# set by the last kernel() call; test harness reads exec_time_ns from here
LAST_RESULTS = None

_BUILD_CACHE = {}


def _bcast(ap, n):
    """Append a stride-0 free dim of size n to an AP (free-dim broadcast)."""
    return AP(ap.tensor, ap.offset, [list(d) for d in ap.ap] + [[0, n]])


def _build_nc(qk_bias: bool, v_bias: bool) -> bass.Bass:
    nc = bacc.Bacc()

    # host pre-arranges all inputs partition-major; xt is chunked per
    # token tile [p][t][k][128] so early v tiles unblock on partial loads
    xt_d = nc.declare_dram_parameter("xt", [P, NT * KC * P], BF16, isOutput=False)
    wqk_d = nc.declare_dram_parameter("wqk", [P, 2 * KC * FG], BF16, isOutput=False)
    wv_d = nc.declare_dram_parameter("wv", [P, KC * FG], BF16, isOutput=False)
    wp_d = nc.declare_dram_parameter("wp", [P, (FG // P) * C], BF16, isOutput=False)
    mstrip_d = nc.declare_dram_parameter("mstrip", [P, 3 * P], BF16, isOutput=False)
    if qk_bias:
        bqk_d = nc.declare_dram_parameter("bqk", [8, P], F32, isOutput=False)
    if v_bias:
        bv_d = nc.declare_dram_parameter("bv", [P, HG * D], BF16, isOutput=False)
    y_d = nc.declare_dram_parameter("y", [T, C], BF16, isOutput=True)

    with tile.TileContext(nc) as tc:
        with tc.tile_pool(name="const", bufs=1) as const, \
             tc.tile_pool(name="stage", bufs=4) as stage_p, \
             tc.tile_pool(name="apack", bufs=3) as apack_p, \
             tc.tile_pool(name="ypool", bufs=6) as y_pool:

            # ---- resident SBUF tiles (merged; sliced via rearrange views)
            xt = const.tile([P, NT * KC * P], BF16, tag="xt", name="xt")
            xtv = xt.rearrange("p (t a f) -> p t a f", a=KC, f=P)
            wqk = const.tile([P, KC * 2 * FG], BF16, tag="wqk", name="wqk")
            wqkv = wqk.rearrange("p (ft a f) -> p ft a f", a=KC, f=P)
            wv = const.tile([P, KC * FG], BF16, tag="wv", name="wv")
            wvv = wv.rearrange("p (a f) -> p a f", f=FG)
            wp = const.tile([P, (FG // P) * C], BF16, tag="wp", name="wp")
            wpv = wp.rearrange("p (a f) -> p a f", f=C)
            qkT = const.tile([P, 8 * T], BF16, tag="qkT", name="qkT")
            qkv_v = qkT.rearrange("p (f t) -> p f t", t=T)
            vag = const.tile([P, NT * HG * (D + 1)], BF16, tag="vag", name="vag")
            vagv = vag.rearrange("p (t h c) -> p t h c", h=HG, c=D + 1)
            aTb = const.tile([P, (FG // P) * T], BF16, tag="aTb", name="aTb")
            aTv = aTb.rearrange("p (g t) -> p g t", t=T)
            # rolling window of 4 band strips per head pair (strips are
            # computed one block ahead of their first PV use); each strip
            # tile holds both heads of the pair (head idx at cols idx*W3)
            e_all = const.tile([P, 4 * 4 * 2 * W3], BF16, tag="e", name="e")
            e_v = e_all.rearrange("p (hp s c) -> p hp s c", s=4, c=2 * W3)
            # [wedge0 | wedge2 | identity]; the middle 128-column chunk of
            # every strip is always fully inside the band
            mask_t = const.tile([P, 3 * P], BF16, tag="mask", name="mask")

            # warm-up fodder that depends on NO input DMA: the PE clock
            # ramp can start ~1us into the kernel
            warm_src = const.tile([P, 2 * P], BF16, tag="warm", name="warm")
            nc.gpsimd.memset(warm_src[:], 1.0)

            # ---- input DMA streams: sync queue feeds wv then the x^T
            # chunks (these gate the v tiles that open phase A), the
            # scalar HWDGE queue streams the q/k/proj weights; the small
            # descriptor-heavy mask load goes last (not needed until the
            # first PV)
            nc.gpsimd.memset(vagv[:, :, :, D:D + 1], 1.0)
            HWV = KC * FG // 2
            nc.sync.dma_start(wv[:, 0:HWV], wv_d[:, 0:HWV])
            nc.scalar.dma_start(wv[:, HWV:2 * HWV], wv_d[:, HWV:2 * HWV])
            XCH = 2 * KC * P            # two token tiles per chunk
            for c in range(NT // 2):
                nc.sync.dma_start(xt[:, c * XCH:(c + 1) * XCH],
                                  xt_d[:, c * XCH:(c + 1) * XCH])
            nc.scalar.dma_start(wqk[:, 0:KC * FG], wqk_d[:, 0:KC * FG])
            nc.scalar.dma_start(wqk[:, KC * FG:2 * KC * FG],
                                wqk_d[:, KC * FG:2 * KC * FG])
            nc.scalar.dma_start(wp[:], wp_d[:])
            nc.scalar.dma_start(mask_t[:], mstrip_d[:])
            if qk_bias:
                bqk_t = const.tile([P, 8], F32, tag="bqk", name="bqk")
                nc.scalar.dma_start(bqk_t[:], bqk_d.rearrange("a p -> p a"))
            if v_bias:
                bv_t = const.tile([P, HG * D], BF16, tag="bv", name="bv")
                nc.scalar.dma_start(bv_t[:], bv_d[:])
            bvv = bv_t.rearrange("p (h c) -> p h c", c=D) if v_bias else None

            with tc.tile_pool(name="ps_qkv", bufs=2, space="PSUM") as ps_qkv, \
                 tc.tile_pool(name="ps_s", bufs=3, space="PSUM") as ps_s, \
                 tc.tile_pool(name="ps_a", bufs=2, space="PSUM") as ps_a, \
                 tc.tile_pool(name="ps_t", bufs=1, space="PSUM") as ps_t:

                # warmup: garbage matmuls on the mask tile ramp the PE
                # clock while the first input DMAs are still in flight
                warm_tile = [None]

                def emit_warm(n):
                    if warm_tile[0] is None:
                        warm_tile[0] = ps_s.tile([P, W3], F32, tag="s",
                                                 name="s")
                    for _ in range(n):
                        nc.tensor.matmul(warm_tile[0][:, 0:2 * P],
                                         lhsT=warm_src[:, 0:P],
                                         rhs=warm_src[:],
                                         start=True, stop=True,
                                         skip_group_check=True)

                emit_warm(NWARM)

                def drain512(dst, ps, late):
                    # drain a [128, 512] psum tile (gpsimd cannot read
                    # PSUM); phase A/B splits with the idle scalar
                    # engine, phase C keeps scalar exp-only
                    if late:
                        nc.vector.tensor_copy(dst[:], ps[:])
                    else:
                        nc.scalar.copy(dst[:, 0:256], ps[:, 0:256])
                        nc.vector.tensor_copy(dst[:, 256:512], ps[:, 256:512])

                def emit_v(t, late=False, warm=0):
                    # token-major [128 tok, FG], into [v_h|1] slots; for
                    # the DMA-paced early tiles, warm matmuls between the
                    # k-steps keep the PE active so HAM holds the clock
                    # up while the inputs trickle in
                    ps = ps_qkv.tile([P, 512], F32, tag="qkv", name="qkv")
                    for k in range(KC):
                        if warm:
                            emit_warm(warm)
                        nc.tensor.matmul(
                            ps[:],
                            lhsT=xtv[:, t, k, :],
                            rhs=wvv[:, k, :],
                            start=(k == 0),
                            stop=(k == KC - 1),
                            skip_group_check=True,
                        )
                    psv = ps.rearrange("p (h c) -> p h c", c=D)
                    if v_bias:
                        nc.vector.tensor_add(
                            vagv[:, t, 0:4, 0:D], psv[:, 0:4, :], bvv[:, 0:4, :])
                        nc.vector.tensor_add(
                            vagv[:, t, 4:8, 0:D], psv[:, 4:8, :], bvv[:, 4:8, :])
                    else:
                        nc.scalar.copy(vagv[:, t, 0:4, 0:D], psv[:, 0:4, :])
                        nc.vector.tensor_copy(vagv[:, t, 4:8, 0:D], psv[:, 4:8, :])

                def emit_qk_tile(ft, nt, late=False):
                    # one [128, 512] output tile of the q/k projection
                    ps = ps_qkv.tile([P, 512], F32, tag="qkv", name="qkv")
                    for k in range(KC):
                        nc.tensor.matmul(
                            ps[:],
                            lhsT=wqkv[:, ft, k, :],
                            rhs=xtv[:, 4 * nt:4 * nt + 4, k, :],
                            start=(k == 0),
                            stop=(k == KC - 1),
                        )
                    dst = qkv_v[:, ft, nt * 512:(nt + 1) * 512]
                    if qk_bias:
                        nc.scalar.activation(
                            dst, ps[:],
                            mybir.ActivationFunctionType.Copy,
                            bias=bqk_t[:, ft:ft + 1],
                        )
                    else:
                        drain512(dst, ps, late)

                proj_yt = {}

                def emit_proj_half(j, n):
                    # half of the output projection for token block j
                    if n == 0:
                        proj_yt[j] = y_pool.tile([P, C], BF16, tag="y", name="y")
                    yt = proj_yt[j]
                    ps2 = ps_qkv.tile([P, 512], F32, tag="qkv", name="qkv")
                    # contract the PE-transposed a^T chunks (2,3) first:
                    # the XBAR chunks (0,1) get extra time in flight
                    for i2, k2 in enumerate((2, 3, 0, 1)):
                        nc.tensor.matmul(
                            ps2[:],
                            lhsT=aTv[:, k2, j * P:(j + 1) * P],
                            rhs=wpv[:, k2, n * 512:(n + 1) * 512],
                            start=(i2 == 0),
                            stop=(i2 == FG // P - 1),
                        )
                    if j == NT - 1:
                        # last block: split drains + per-half y DMA
                        # shorten the tail chain
                        o = n * 512
                        nc.scalar.copy(yt[:, o:o + 256], ps2[:, 0:256])
                        nc.vector.tensor_copy(yt[:, o + 256:o + 512],
                                              ps2[:, 256:512])
                        nc.sync.dma_start(
                            y_d[j * P:(j + 1) * P, o:o + 512],
                            yt[:, o:o + 512])
                        if n == 1:
                            del proj_yt[j]
                    else:
                        drain512(yt[:, n * 512:(n + 1) * 512], ps2, True)
                        if n == 1:
                            nc.sync.dma_start(y_d[j * P:(j + 1) * P, :], yt[:])
                            del proj_yt[j]

                def _chunk_multi(hp0, npair, s, off):
                    # view chunk [off:off+128] of BOTH heads of npair
                    # consecutive pair-strip tiles as [P, npair, 2, 128]
                    base = e_v[:, hp0, s, off:off + P]
                    return AP(base.tensor, base.offset,
                              [list(base.ap[0]), [4 * 2 * W3, npair],
                               [W3, 2], [1, P]])

                def _mask_bc(npair, off):
                    # one wedge of the mask, broadcast across pairs/heads
                    base = mask_t[:, off:off + P]
                    return AP(base.tensor, base.offset,
                              [list(base.ap[0]), [0, npair], [0, 2], [1, P]])

                def emit_strip(j, hp):
                    # band strip for key block j, one head pair: S^T via
                    # [64x128]^T @ [64, w] (the odd head's q/k live at
                    # partitions 64..127 so the two K=64 matmuls run in
                    # disjoint PE row groups), Exp on scalar.  Only the
                    # dj=0 wedge is masked here (vector, on the critical
                    # path to this block's PV); the dj=2 wedge isn't read
                    # until block j+2 and is masked off-path on gpsimd.
                    w = min(W3, (NT - j) * P)
                    e_dst = e_v[:, hp, j % 4, :]
                    for idx in range(2):
                        ho = idx * D
                        ps = ps_s.tile([P, W3], F32, tag="s", name="s")
                        nc.tensor.matmul(
                            ps[:, :w],
                            lhsT=qkv_v[ho:ho + D, 4 + hp, j * P:(j + 1) * P],
                            rhs=qkv_v[ho:ho + D, hp, j * P:j * P + w],
                            start=True, stop=True,
                        )
                        nc.scalar.activation(
                            e_dst[:, idx * W3:idx * W3 + w], ps[:, :w],
                            mybir.ActivationFunctionType.Exp,
                            scale=0.125,
                        )

                def emit_wedge0(j, pp):
                    # dj=0 wedge mask for two pair-strips in one op
                    c0 = _chunk_multi(2 * pp, 2, j % 4, 0)
                    nc.gpsimd.tensor_mul(c0, c0, _mask_bc(2, 0))

                def emit_wedge2(i):
                    # deferred dj=2 wedge mask for strip i, all 4 pairs in
                    # one op (read at block i+2; emitted during block i+1)
                    c2 = _chunk_multi(0, 4, i % 4, 2 * P)
                    nc.gpsimd.tensor_mul(c2, c2, _mask_bc(4, P))

                def emit_pv(j, pp, a_pack):
                    # query-major PV for TWO head pairs (4 heads) into one
                    # psum tile [128 tq, 4*(65)]; cols 64/129/194/259 are
                    # the softmax denominators.  Drain = one reciprocal
                    # [128,4] + ONE broadcast multiply into the packed a
                    # tile (stride-0 AP replicates 1/s across the 64
                    # feature columns of each head).
                    pa = ps_a.tile([P, 4 * (D + 1)], F32, tag="psA", name="psA")
                    pav = pa.rearrange("p (h c) -> p h c", c=D + 1)
                    i0 = max(0, j - 2)
                    for hpi in range(2):
                        hp = 2 * pp + hpi
                        for i in range(i0, j + 1):
                            dj = j - i
                            for idx in range(2):
                                h = 2 * hp + idx
                                o = 2 * hpi + idx
                                nc.tensor.matmul(
                                    pa[:, o * (D + 1):(o + 1) * (D + 1)],
                                    lhsT=e_v[:, hp, i % 4,
                                             idx * W3 + dj * P:
                                             idx * W3 + (dj + 1) * P],
                                    rhs=vagv[:, i, h, :],
                                    start=(hpi == 0 and i == i0 and idx == 0),
                                    stop=(hpi == 1 and i == j and idx == 1),
                                    skip_group_check=True,
                                )
                    rs = stage_p.tile([P, 4], F32, tag="rs", name="rs")
                    nc.vector.reciprocal(
                        rs.rearrange("p (h c) -> p h c", c=1),
                        pav[:, :, D:D + 1],
                    )
                    nc.vector.tensor_mul(
                        a_pack[:, pp * 2 * P:(pp + 1) * 2 * P].rearrange(
                            "p (h c) -> p h c", c=D),
                        pav[:, :, 0:D],
                        _bcast(rs[:], D),
                    )

                def emit_transposes(j, a_pack):
                    # a^T for the 4 feature blocks: 2 on the sync XBAR
                    # (which carries nothing else in phase C) and 2 on
                    # the PE (is_transpose matmuls into a bf16 PSUM tile,
                    # drained by vector).  The final blocks run all 4 on
                    # the PE: their projections follow at lag <= 1, and
                    # the PE path is ready ~2us sooner than the XBAR.
                    all_pe = j >= NT - 6
                    hps = (0, 1, 2, 3) if all_pe else (2, 3)
                    if not all_pe:
                        for hp in range(2):
                            nc.sync.dma_start_transpose(
                                aTv[:, hp, j * P:(j + 1) * P],
                                a_pack[:, hp * P:(hp + 1) * P],
                            )
                    pst = ps_t.tile([P, 4 * P], BF16, tag="aT", name="aT")
                    for n, hp in enumerate(hps):
                        nc.tensor.matmul(
                            pst[:, n * P:(n + 1) * P],
                            lhsT=a_pack[:, hp * P:(hp + 1) * P],
                            rhs=mask_t[:, 2 * P:3 * P],
                            is_transpose=True,
                            start=True, stop=True,
                            skip_group_check=True,
                        )
                    pstv = pst.rearrange("p (g t) -> p g t", t=P)
                    if all_pe:
                        # the next proj contracts chunks (2,3) first, so
                        # drain them first on vector; scalar (idle in the
                        # late blocks) takes (0,1) in parallel
                        nc.vector.tensor_copy(
                            aTv[:, 2:4, j * P:(j + 1) * P], pstv[:, 2:4, :])
                        nc.scalar.copy(
                            aTv[:, 0:2, j * P:(j + 1) * P], pstv[:, 0:2, :])
                    else:
                        nc.vector.tensor_copy(
                            aTv[:, 2:4, j * P:(j + 1) * P], pstv[:, 0:2, :])

                # phase A: all v tiles first -- they are gated only on wv
                # and the fine-grained x^T chunks, which land first, so
                # the PE start never depends on the (larger) wqk loads;
                # v12-15 are deferred into phase C
                for t in range(12):
                    emit_v(t)
                for ft in range(8):
                    emit_qk_tile(ft, 0)
                for ft in range(8):
                    emit_qk_tile(ft, 1)

                # phase C: query blocks, each with dense filler units
                # (remaining q/k tiles, deferred v tiles, and the output
                # projection of earlier blocks) so the PE always sees big
                # matmuls and HAM holds the clock at 2.4 GHz.
                def qk_f(ft, nt):
                    return lambda: emit_qk_tile(ft, nt, True)

                def v_f(t):
                    return lambda: emit_v(t, True)

                def pr_f(j):
                    return lambda: (emit_proj_half(j, 0), emit_proj_half(j, 1))

                # proj lags its attention block by 2 (the PE transposes
                # drain during the same block); qk (ft,2) done by block 5,
                # (ft,3) by block 10; v tile t by block t-1
                fill_sched = {
                    0: [qk_f(0, 2), qk_f(1, 2)],
                    1: [qk_f(2, 2), qk_f(3, 2)],
                    2: [qk_f(4, 2), pr_f(0)],
                    3: [qk_f(5, 2), pr_f(1)],
                    4: [qk_f(6, 2), pr_f(2)],
                    5: [qk_f(7, 2), pr_f(3)],
                    6: [qk_f(0, 3), pr_f(4)],
                    7: [qk_f(1, 3), pr_f(5)],
                    8: [qk_f(2, 3), qk_f(3, 3), pr_f(6)],
                    9: [qk_f(4, 3), qk_f(5, 3), pr_f(7)],
                    10: [qk_f(6, 3), qk_f(7, 3), pr_f(8)],
                    11: [v_f(12), pr_f(9), pr_f(10)],
                    12: [v_f(13), pr_f(11)],
                    13: [v_f(14), pr_f(12)],
                    14: [v_f(15), pr_f(13), pr_f(14)],
                    15: [],
                }
                # strips are produced one block ahead of their first PV
                # consumer, so the S->Exp->wedge0 chain never sits on the
                # critical path of a block
                emit_strip(0, 0)
                emit_strip(0, 1)
                emit_wedge0(0, 0)
                emit_strip(0, 2)
                emit_strip(0, 3)
                emit_wedge0(0, 1)
                for j in range(NT):
                    fillers = fill_sched[j]
                    a_pack = apack_p.tile([P, FG], BF16, tag="ap", name="ap")
                    if j + 1 < NT:
                        emit_strip(j + 1, 0)
                        emit_strip(j + 1, 1)
                        emit_wedge0(j + 1, 0)
                    emit_pv(j, 0, a_pack)
                    if fillers:
                        fillers[0]()
                    if j + 1 < NT:
                        emit_strip(j + 1, 2)
                        emit_strip(j + 1, 3)
                        emit_wedge0(j + 1, 1)
                    emit_pv(j, 1, a_pack)
                    # a filler between the last PV and the transposes
                    # hides the reciprocal->normalize vector chain
                    if len(fillers) > 1:
                        fillers[1]()
                    emit_transposes(j, a_pack)
                    if 1 <= j <= NT - 2:
                        emit_wedge2(j - 1)
                    for f in fillers[2:]:
                        f()
                emit_proj_half(NT - 1, 0)
                emit_proj_half(NT - 1, 1)

    nc.finalize()
    return nc


def _band_mask_strip() -> np.ndarray:
    # compact wedge mask [wedge0 | wedge2 | identity]: wedge0 masks the
    # dj=0 chunk (valid iff tq >= tk), wedge2 the dj=2 chunk (valid iff
    # tq < tk); the identity feeds the PE is_transpose matmuls
    tk = np.arange(P)[:, None]
    tq = np.arange(P)[None, :]
    w0 = (tq >= tk)
    w2 = (tq < tk)
    ident = np.eye(P, dtype=np.float32)
    return np.concatenate([w0, w2, ident], axis=1).astype(ml_dtypes.bfloat16)


def kernel(x, W_attn, b_attn, W_proj, b_proj):
    global LAST_RESULTS
    x = np.asarray(x, dtype=np.float32)
    W_attn = np.asarray(W_attn, dtype=np.float32)
    b_attn = np.asarray(b_attn, dtype=np.float32)
    W_proj = np.asarray(W_proj, dtype=np.float32)
    b_proj = np.asarray(b_proj, dtype=np.float32)

    qk_bias = bool(np.any(b_attn[:2 * C]))
    v_bias = bool(np.any(b_attn[2 * C:]))

    key = (qk_bias, v_bias)
    if key not in _BUILD_CACHE:
        _BUILD_CACHE[key] = _build_nc(qk_bias, v_bias)
    nc = _BUILD_CACHE[key]

    mstrip = _band_mask_strip()
    in_maps = []
    for c in range(8):
        b, g = c // 2, c % 2
        fsl = slice(FG * g, FG * (g + 1))
        # partition-major host layouts so every device load is one big
        # contiguous DMA: xt [p][t,k,f], wqk [p][half,k,f], wv [p][k,f],
        # wp [p][k,c]
        xt_h = (x[b].T.reshape(KC, P, NT, P).transpose(1, 2, 0, 3)
                .reshape(P, NT * KC * P))
        # wqk laid out per q/k feature tile: [p][ft 8][k 8][128]
        wqk_h = (np.concatenate(
            [W_attn[:, fsl], W_attn[:, C + FG * g:C + FG * (g + 1)]], axis=1)
            .reshape(KC, P, 8, P).transpose(1, 2, 0, 3)
            .reshape(P, 2 * KC * FG))
        wv_h = (W_attn[:, 2 * C + FG * g:2 * C + FG * (g + 1)]
                .reshape(KC, P, FG).transpose(1, 0, 2).reshape(P, KC * FG))
        wp_h = (W_proj[fsl, :].reshape(FG // P, P, C).transpose(1, 0, 2)
                .reshape(P, (FG // P) * C))
        im = {
            "xt": np.ascontiguousarray(xt_h).astype(ml_dtypes.bfloat16),
            "wqk": np.ascontiguousarray(wqk_h).astype(ml_dtypes.bfloat16),
            "wv": np.ascontiguousarray(wv_h).astype(ml_dtypes.bfloat16),
            "wp": np.ascontiguousarray(wp_h).astype(ml_dtypes.bfloat16),
            "mstrip": mstrip,
        }
        if qk_bias:
            bq = b_attn[fsl]
            bk = b_attn[C + FG * g:C + FG * (g + 1)]
            im["bqk"] = np.concatenate([bq, bk]).reshape(8, P).astype(np.float32)
        if v_bias:
            bv = b_attn[2 * C + FG * g:2 * C + FG * (g + 1)]
            im["bv"] = np.broadcast_to(
                bv.reshape(1, HG * D), (P, HG * D)
            ).astype(ml_dtypes.bfloat16)
        in_maps.append(im)

    res = run_bass_kernel_spmd(nc, in_maps, list(range(8)))
    LAST_RESULTS = res

    out = np.empty((B, T, C), dtype=np.float32)
    for b in range(B):
        out[b] = (res.results[2 * b]["y"].astype(np.float32)
                  + res.results[2 * b + 1]["y"].astype(np.float32)
                  + b_proj)
    return out


# revision 55
# speedup vs baseline: 1.0135x; 1.0135x over previous
"""Trainium2 Bass kernel for banded (local-causal) multi-head self-attention.

Problem (hardcoded shapes): x [4, 2048, 1024], W_attn [1024, 3072],
b_attn [3072], W_proj [1024, 1024], b_proj [1024]; 16 heads, head dim 64,
local causal window 256.

Sharding over 8 NeuronCores: data-parallel over the 4 batches x
tensor-parallel over 2 head-groups (8 heads each). Each core computes a
partial projection output [2048, 1024]; the host sums the two head-group
partials per batch and adds b_proj.

Per-core device program (all loops fully unrolled under Tile):
  inputs: DMA queues drain serially per queue and share HBM between
          queues, so both queues front-load what phase A consumes
          first: a wv half each, then the x^T chunks on sync and the
          wqk/wp/mask weights on scalar.  ~40 warm matmuls on a memset
          tile (no DMA dependency) ramp the PE clock from ~1us so HAM
          never sees an idle start.
  phase A/B: all v tiles first (token-major [v_h|1] slots of stride 65,
          so PV emits the softmax denominator in its last column) --
          gated only on wv + x^T chunks -- then the q^T/k^T projection
          tiles (feature-major [feat, T]).
  phase C: per query block j: band-strip S^T matmuls for key block j+1
          (Exp on scalar; wedge masks merged into wide 4D-AP gpsimd
          ops), PV for two head pairs per PSUM tile (query-major
          [128, 4*65]; drain = one reciprocal + one stride-0-broadcast
          multiply into a packed a tile), a filler unit to hide the
          reciprocal->normalize vector chain, then the four [128,128]
          feature-block transposes: 2 on the sync XBAR + 2 on the PE
          (is_transpose matmuls into a bf16 PSUM tile) for early
          blocks; ALL 4 on the PE for blocks >= 10 whose projections
          follow at lag <= 1 (the PE path is ready ~2us sooner than the
          XBAR, which also carries the y output DMAs).  Dense filler
          (remaining q/k tiles, deferred v tiles, output projection at
          lag 2) keeps the PE issue-busy >98% so HAM holds 2.4 GHz; the
          last block splits its proj drains across scalar+vector and
          its y DMA into halves to shorten the final chain.

Matmul dtypes: bf16 everywhere (inputs pre-cast on host), fp32 PSUM accum.
"""

import numpy as np
import ml_dtypes

import concourse.bass as bass
import concourse.bacc as bacc
import concourse.mybir as mybir
import concourse.tile as tile
from concourse.bass import AP
from concourse.bass_utils import run_bass_kernel_spmd

B, T, C = 4, 2048, 1024
H, D, CTX = 16, 64, 256
HG = 8                 # heads per core
FG = HG * D            # 512 features per group
P = 128
NT = T // P            # 16 token blocks
KC = C // P            # 8 contraction tiles of C
W3 = 3 * P             # strip width 384
NWARM = 42             # PE clock-ramp warmup matmuls

BF16 = mybir.dt.bfloat16
F32 = mybir.dt.float32

# set by the last kernel() call; test harness reads exec_time_ns from here
LAST_RESULTS = None

_BUILD_CACHE = {}


def _bcast(ap, n):
    """Append a stride-0 free dim of size n to an AP (free-dim broadcast)."""
    return AP(ap.tensor, ap.offset, [list(d) for d in ap.ap] + [[0, n]])


def _build_nc(qk_bias: bool, v_bias: bool) -> bass.Bass:
    nc = bacc.Bacc()

    # host pre-arranges all inputs partition-major; xt is chunked per
    # token tile [p][t][k][128] so early v tiles unblock on partial loads
    xt_d = nc.declare_dram_parameter("xt", [P, NT * KC * P], BF16, isOutput=False)
    wqk_d = nc.declare_dram_parameter("wqk", [P, 2 * KC * FG], BF16, isOutput=False)
    wv_d = nc.declare_dram_parameter("wv", [P, KC * FG], BF16, isOutput=False)
    wp_d = nc.declare_dram_parameter("wp", [P, (FG // P) * C], BF16, isOutput=False)
    mstrip_d = nc.declare_dram_parameter("mstrip", [P, 3 * P], BF16, isOutput=False)
    if qk_bias:
        bqk_d = nc.declare_dram_parameter("bqk", [8, P], F32, isOutput=False)
    if v_bias:
        bv_d = nc.declare_dram_parameter("bv", [P, HG * D], BF16, isOutput=False)
    y_d = nc.declare_dram_parameter("y", [T, C], BF16, isOutput=True)

    with tile.TileContext(nc) as tc:
        with tc.tile_pool(name="const", bufs=1) as const, \
             tc.tile_pool(name="stage", bufs=4) as stage_p, \
             tc.tile_pool(name="apack", bufs=3) as apack_p, \
             tc.tile_pool(name="ypool", bufs=6) as y_pool:

            # ---- resident SBUF tiles (merged; sliced via rearrange views)
            xt = const.tile([P, NT * KC * P], BF16, tag="xt", name="xt")
            xtv = xt.rearrange("p (t a f) -> p t a f", a=KC, f=P)
            wqk = const.tile([P, KC * 2 * FG], BF16, tag="wqk", name="wqk")
            wqkv = wqk.rearrange("p (ft a f) -> p ft a f", a=KC, f=P)
            wv = const.tile([P, KC * FG], BF16, tag="wv", name="wv")
            wvv = wv.rearrange("p (a f) -> p a f", f=FG)
            wp = const.tile([P, (FG // P) * C], BF16, tag="wp", name="wp")
            wpv = wp.rearrange("p (a f) -> p a f", f=C)
            qkT = const.tile([P, 8 * T], BF16, tag="qkT", name="qkT")
            qkv_v = qkT.rearrange("p (f t) -> p f t", t=T)
            vag = const.tile([P, NT * HG * (D + 1)], BF16, tag="vag", name="vag")
            vagv = vag.rearrange("p (t h c) -> p t h c", h=HG, c=D + 1)
            aTb = const.tile([P, (FG // P) * T], BF16, tag="aTb", name="aTb")
            aTv = aTb.rearrange("p (g t) -> p g t", t=T)
            # rolling window of 4 band strips per head pair (strips are
            # computed one block ahead of their first PV use); each strip
            # tile holds both heads of the pair (head idx at cols idx*W3)
            e_all = const.tile([P, 4 * 4 * 2 * W3], BF16, tag="e", name="e")
            e_v = e_all.rearrange("p (hp s c) -> p hp s c", s=4, c=2 * W3)
            # [wedge0 | wedge2 | identity]; the middle 128-column chunk of
            # every strip is always fully inside the band
            mask_t = const.tile([P, 3 * P], BF16, tag="mask", name="mask")

            # warm-up fodder that depends on NO input DMA: the PE clock
            # ramp can start ~1us into the kernel
            warm_src = const.tile([P, 2 * P], BF16, tag="warm", name="warm")
            nc.gpsimd.memset(warm_src[:], 1.0)

            # ---- input DMA streams: sync queue feeds wv then the x^T
            # chunks (these gate the v tiles that open phase A), the
            # scalar HWDGE queue streams the q/k/proj weights; the small
            # descriptor-heavy mask load goes last (not needed until the
            # first PV)
            nc.gpsimd.memset(vagv[:, :, :, D:D + 1], 1.0)
            HWV = KC * FG // 2
            nc.sync.dma_start(wv[:, 0:HWV], wv_d[:, 0:HWV])
            nc.scalar.dma_start(wv[:, HWV:2 * HWV], wv_d[:, HWV:2 * HWV])
            XCH = 2 * KC * P            # two token tiles per chunk
            for c in range(NT // 2):
                nc.sync.dma_start(xt[:, c * XCH:(c + 1) * XCH],
                                  xt_d[:, c * XCH:(c + 1) * XCH])
            nc.scalar.dma_start(wqk[:, 0:KC * FG], wqk_d[:, 0:KC * FG])
            nc.scalar.dma_start(wqk[:, KC * FG:2 * KC * FG],
                                wqk_d[:, KC * FG:2 * KC * FG])
            nc.scalar.dma_start(wp[:], wp_d[:])
            nc.scalar.dma_start(mask_t[:], mstrip_d[:])
            if qk_bias:
                bqk_t = const.tile([P, 8], F32, tag="bqk", name="bqk")
                nc.scalar.dma_start(bqk_t[:], bqk_d.rearrange("a p -> p a"))
            if v_bias:
                bv_t = const.tile([P, HG * D], BF16, tag="bv", name="bv")
                nc.scalar.dma_start(bv_t[:], bv_d[:])
            bvv = bv_t.rearrange("p (h c) -> p h c", c=D) if v_bias else None

            with tc.tile_pool(name="ps_qkv", bufs=2, space="PSUM") as ps_qkv, \
                 tc.tile_pool(name="ps_s", bufs=3, space="PSUM") as ps_s, \
                 tc.tile_pool(name="ps_a", bufs=2, space="PSUM") as ps_a, \
                 tc.tile_pool(name="ps_t", bufs=1, space="PSUM") as ps_t:

                # warmup: garbage matmuls on the mask tile ramp the PE
                # clock while the first input DMAs are still in flight
                warm_tile = [None]

                def emit_warm(n):
                    if warm_tile[0] is None:
                        warm_tile[0] = ps_s.tile([P, W3], F32, tag="s",
                                                 name="s")
                    for _ in range(n):
                        nc.tensor.matmul(warm_tile[0][:, 0:2 * P],
                                         lhsT=warm_src[:, 0:P],
                                         rhs=warm_src[:],
                                         start=True, stop=True,
                                         skip_group_check=True)

                emit_warm(NWARM)

                def drain512(dst, ps, late):
                    # drain a [128, 512] psum tile (gpsimd cannot read
                    # PSUM); phase A/B splits with the idle scalar
                    # engine, phase C keeps scalar exp-only
                    if late:
                        nc.vector.tensor_copy(dst[:], ps[:])
                    else:
                        nc.scalar.copy(dst[:, 0:256], ps[:, 0:256])
                        nc.vector.tensor_copy(dst[:, 256:512], ps[:, 256:512])

                def emit_v(t, late=False, warm=0):
                    # token-major [128 tok, FG], into [v_h|1] slots; for
                    # the DMA-paced early tiles, warm matmuls between the
                    # k-steps keep the PE active so HAM holds the clock
                    # up while the inputs trickle in
                    ps = ps_qkv.tile([P, 512], F32, tag="qkv", name="qkv")
                    for k in range(KC):
                        if warm:
                            emit_warm(warm)
                        nc.tensor.matmul(
                            ps[:],
                            lhsT=xtv[:, t, k, :],
                            rhs=wvv[:, k, :],
                            start=(k == 0),
                            stop=(k == KC - 1),
                            skip_group_check=True,
                        )
                    psv = ps.rearrange("p (h c) -> p h c", c=D)
                    if v_bias:
                        nc.vector.tensor_add(
                            vagv[:, t, 0:4, 0:D], psv[:, 0:4, :], bvv[:, 0:4, :])
                        nc.vector.tensor_add(
                            vagv[:, t, 4:8, 0:D], psv[:, 4:8, :], bvv[:, 4:8, :])
                    else:
                        nc.scalar.copy(vagv[:, t, 0:4, 0:D], psv[:, 0:4, :])
                        nc.vector.tensor_copy(vagv[:, t, 4:8, 0:D], psv[:, 4:8, :])

                def emit_qk_tile(ft, nt, late=False):
                    # one [128, 512] output tile of the q/k projection
                    ps = ps_qkv.tile([P, 512], F32, tag="qkv", name="qkv")
                    for k in range(KC):
                        nc.tensor.matmul(
                            ps[:],
                            lhsT=wqkv[:, ft, k, :],
                            rhs=xtv[:, 4 * nt:4 * nt + 4, k, :],
                            start=(k == 0),
                            stop=(k == KC - 1),
                        )
                    dst = qkv_v[:, ft, nt * 512:(nt + 1) * 512]
                    if qk_bias:
                        nc.scalar.activation(
                            dst, ps[:],
                            mybir.ActivationFunctionType.Copy,
                            bias=bqk_t[:, ft:ft + 1],
                        )
                    else:
                        drain512(dst, ps, late)

                proj_yt = {}

                def emit_proj_half(j, n):
                    # half of the output projection for token block j
                    if n == 0:
                        proj_yt[j] = y_pool.tile([P, C], BF16, tag="y", name="y")
                    yt = proj_yt[j]
                    ps2 = ps_qkv.tile([P, 512], F32, tag="qkv", name="qkv")
                    # contract the PE-transposed a^T chunks (2,3) first:
                    # the XBAR chunks (0,1) get extra time in flight
                    for i2, k2 in enumerate((2, 3, 0, 1)):
                        nc.tensor.matmul(
                            ps2[:],
                            lhsT=aTv[:, k2, j * P:(j + 1) * P],
                            rhs=wpv[:, k2, n * 512:(n + 1) * 512],
                            start=(i2 == 0),
                            stop=(i2 == FG // P - 1),
                        )
                    if j == NT - 1:
                        # last block: split drains + per-half y DMA
                        # shorten the tail chain
                        o = n * 512
                        nc.scalar.copy(yt[:, o:o + 256], ps2[:, 0:256])
                        nc.vector.tensor_copy(yt[:, o + 256:o + 512],
                                              ps2[:, 256:512])
                        nc.sync.dma_start(
                            y_d[j * P:(j + 1) * P, o:o + 512],
                            yt[:, o:o + 512])
                        if n == 1:
                            del proj_yt[j]
                    else:
                        drain512(yt[:, n * 512:(n + 1) * 512], ps2, True)
                        if n == 1:
                            nc.sync.dma_start(y_d[j * P:(j + 1) * P, :], yt[:])
                            del proj_yt[j]

                def _chunk_multi(hp0, npair, s, off):
                    # view chunk [off:off+128] of BOTH heads of npair
                    # consecutive pair-strip tiles as [P, npair, 2, 128]
                    base = e_v[:, hp0, s, off:off + P]
                    return AP(base.tensor, base.offset,
                              [list(base.ap[0]), [4 * 2 * W3, npair],
                               [W3, 2], [1, P]])

                def _mask_bc(npair, off):
                    # one wedge of the mask, broadcast across pairs/heads
                    base = mask_t[:, off:off + P]
                    return AP(base.tensor, base.offset,
                              [list(base.ap[0]), [0, npair], [0, 2], [1, P]])

                def emit_strip(j, hp):
                    # band strip for key block j, one head pair: S^T via
                    # [64x128]^T @ [64, w] (the odd head's q/k live at
                    # partitions 64..127 so the two K=64 matmuls run in
                    # disjoint PE row groups), Exp on scalar.  Only the
                    # dj=0 wedge is masked here (vector, on the critical
                    # path to this block's PV); the dj=2 wedge isn't read
                    # until block j+2 and is masked off-path on gpsimd.
                    w = min(W3, (NT - j) * P)
                    e_dst = e_v[:, hp, j % 4, :]
                    for idx in range(2):
                        ho = idx * D
                        ps = ps_s.tile([P, W3], F32, tag="s", name="s")
                        nc.tensor.matmul(
                            ps[:, :w],
                            lhsT=qkv_v[ho:ho + D, 4 + hp, j * P:(j + 1) * P],
                            rhs=qkv_v[ho:ho + D, hp, j * P:j * P + w],
                            start=True, stop=True,
                        )
                        nc.scalar.activation(
                            e_dst[:, idx * W3:idx * W3 + w], ps[:, :w],
                            mybir.ActivationFunctionType.Exp,
                            scale=0.125,
                        )

                def emit_wedge0(j, pp):
                    # dj=0 wedge mask for two pair-strips in one op
                    c0 = _chunk_multi(2 * pp, 2, j % 4, 0)
                    nc.gpsimd.tensor_mul(c0, c0, _mask_bc(2, 0))

                def emit_wedge2(i):
                    # deferred dj=2 wedge mask for strip i, all 4 pairs in
                    # one op (read at block i+2; emitted during block i+1)
                    c2 = _chunk_multi(0, 4, i % 4, 2 * P)
                    nc.gpsimd.tensor_mul(c2, c2, _mask_bc(4, P))

                def emit_pv(j, pp, a_pack):
                    # query-major PV for TWO head pairs (4 heads) into one
                    # psum tile [128 tq, 4*(65)]; cols 64/129/194/259 are
                    # the softmax denominators.  Drain = one reciprocal
                    # [128,4] + ONE broadcast multiply into the packed a
                    # tile (stride-0 AP replicates 1/s across the 64
                    # feature columns of each head).
                    pa = ps_a.tile([P, 4 * (D + 1)], F32, tag="psA", name="psA")
                    pav = pa.rearrange("p (h c) -> p h c", c=D + 1)
                    i0 = max(0, j - 2)
                    for hpi in range(2):
                        hp = 2 * pp + hpi
                        for i in range(i0, j + 1):
                            dj = j - i
                            for idx in range(2):
                                h = 2 * hp + idx
                                o = 2 * hpi + idx
                                nc.tensor.matmul(
                                    pa[:, o * (D + 1):(o + 1) * (D + 1)],
                                    lhsT=e_v[:, hp, i % 4,
                                             idx * W3 + dj * P:
                                             idx * W3 + (dj + 1) * P],
                                    rhs=vagv[:, i, h, :],
                                    start=(hpi == 0 and i == i0 and idx == 0),
                                    stop=(hpi == 1 and i == j and idx == 1),
                                    skip_group_check=True,
                                )
                    rs = stage_p.tile([P, 4], F32, tag="rs", name="rs")
                    nc.vector.reciprocal(
                        rs.rearrange("p (h c) -> p h c", c=1),
                        pav[:, :, D:D + 1],
                    )
                    nc.vector.tensor_mul(
                        a_pack[:, pp * 2 * P:(pp + 1) * 2 * P].rearrange(
                            "p (h c) -> p h c", c=D),
                        pav[:, :, 0:D],
                        _bcast(rs[:], D),
                    )

                def emit_transposes(j, a_pack):
                    # a^T for the 4 feature blocks: 2 on the sync XBAR
                    # (which carries nothing else in phase C) and 2 on
                    # the PE (is_transpose matmuls into a bf16 PSUM tile,
                    # drained by vector).  The final blocks run all 4 on
                    # the PE: their projections follow at lag <= 1, and
                    # the PE path is ready ~2us sooner than the XBAR.
                    all_pe = j >= NT - 6
                    hps = (0, 1, 2, 3) if all_pe else (2, 3)
                    if not all_pe:
                        for hp in range(2):
                            nc.sync.dma_start_transpose(
                                aTv[:, hp, j * P:(j + 1) * P],
                                a_pack[:, hp * P:(hp + 1) * P],
                            )
                    pst = ps_t.tile([P, 4 * P], BF16, tag="aT", name="aT")
                    for n, hp in enumerate(hps):
                        nc.tensor.matmul(
                            pst[:, n * P:(n + 1) * P],
                            lhsT=a_pack[:, hp * P:(hp + 1) * P],
                            rhs=mask_t[:, 2 * P:3 * P],
                            is_transpose=True,
                            start=True, stop=True,
                            skip_group_check=True,
                        )
                    pstv = pst.rearrange("p (g t) -> p g t", t=P)
                    if all_pe:
                        # the next proj contracts chunks (2,3) first, so
                        # drain them first on vector; scalar (idle in the
                        # late blocks) takes (0,1) in parallel
                        nc.vector.tensor_copy(
                            aTv[:, 2:4, j * P:(j + 1) * P], pstv[:, 2:4, :])
                        nc.scalar.copy(
                            aTv[:, 0:2, j * P:(j + 1) * P], pstv[:, 0:2, :])
                    else:
                        nc.vector.tensor_copy(
                            aTv[:, 2:4, j * P:(j + 1) * P], pstv[:, 0:2, :])

                # phase A: all v tiles first -- they are gated only on wv
                # and the fine-grained x^T chunks, which land first, so
                # the PE start never depends on the (larger) wqk loads;
                # v12-15 are deferred into phase C
                for t in range(12):
                    emit_v(t)
                for ft in range(8):
                    emit_qk_tile(ft, 0)
                for ft in range(8):
                    emit_qk_tile(ft, 1)

                # phase C: query blocks, each with dense filler units
                # (remaining q/k tiles, deferred v tiles, and the output
                # projection of earlier blocks) so the PE always sees big
                # matmuls and HAM holds the clock at 2.4 GHz.
                def qk_f(ft, nt):
                    return lambda: emit_qk_tile(ft, nt, True)

                def v_f(t):
                    return lambda: emit_v(t, True)

                def pr_f(j):
                    return lambda: (emit_proj_half(j, 0), emit_proj_half(j, 1))

                # proj lags its attention block by 2 (the PE transposes
                # drain during the same block); qk (ft,2) done by block 5,
                # (ft,3) by block 10; v tile t by block t-1
                fill_sched = {
                    0: [qk_f(0, 2), qk_f(1, 2)],
                    1: [qk_f(2, 2), qk_f(3, 2)],
                    2: [qk_f(4, 2), pr_f(0)],
                    3: [qk_f(5, 2), pr_f(1)],
                    4: [qk_f(6, 2), pr_f(2)],
                    5: [qk_f(7, 2), pr_f(3)],
                    6: [qk_f(0, 3), pr_f(4)],
                    7: [qk_f(1, 3), pr_f(5)],
                    8: [qk_f(2, 3), qk_f(3, 3), pr_f(6)],
                    9: [qk_f(4, 3), qk_f(5, 3), pr_f(7)],
                    10: [qk_f(6, 3), qk_f(7, 3), pr_f(8)],
                    11: [v_f(12), pr_f(9), pr_f(10)],
                    12: [v_f(13), pr_f(11)],
                    13: [v_f(14), pr_f(12)],
                    14: [v_f(15), pr_f(13)],
                    15: [],
                }
                # strips are produced one block ahead of their first PV
                # consumer, so the S->Exp->wedge0 chain never sits on the
                # critical path of a block
                emit_strip(0, 0)
                emit_strip(0, 1)
                emit_wedge0(0, 0)
                emit_strip(0, 2)
                emit_strip(0, 3)
                emit_wedge0(0, 1)
                for j in range(NT):
                    fillers = fill_sched[j]
                    a_pack = apack_p.tile([P, FG], BF16, tag="ap", name="ap")
                    if j + 1 < NT:
                        emit_strip(j + 1, 0)
                        emit_strip(j + 1, 1)
                        emit_wedge0(j + 1, 0)
                    emit_pv(j, 0, a_pack)
                    if fillers:
                        fillers[0]()
                    if j + 1 < NT:
                        emit_strip(j + 1, 2)
                        emit_strip(j + 1, 3)
                        emit_wedge0(j + 1, 1)
                    emit_pv(j, 1, a_pack)
                    # a filler between the last PV and the transposes
                    # hides the reciprocal->normalize vector chain; the
                    # final block hides it under the previous block's proj
                    if j == NT - 1:
                        emit_proj_half(NT - 2, 0)
                        emit_proj_half(NT - 2, 1)
                    if len(fillers) > 1:
                        fillers[1]()
                    emit_transposes(j, a_pack)
                    if 1 <= j <= NT - 2:
                        emit_wedge2(j - 1)
                    for f in fillers[2:]:
                        f()
                emit_proj_half(NT - 1, 0)
                emit_proj_half(NT - 1, 1)

    nc.finalize()
    return nc


def _band_mask_strip() -> np.ndarray:
    # compact wedge mask [wedge0 | wedge2 | identity]: wedge0 masks the
    # dj=0 chunk (valid iff tq >= tk), wedge2 the dj=2 chunk (valid iff
    # tq < tk); the identity feeds the PE is_transpose matmuls
    tk = np.arange(P)[:, None]
    tq = np.arange(P)[None, :]
    w0 = (tq >= tk)
    w2 = (tq < tk)
    ident = np.eye(P, dtype=np.float32)
    return np.concatenate([w0, w2, ident], axis=1).astype(ml_dtypes.bfloat16)


def kernel(x, W_attn, b_attn, W_proj, b_proj):
    global LAST_RESULTS
    x = np.asarray(x, dtype=np.float32)
    W_attn = np.asarray(W_attn, dtype=np.float32)
    b_attn = np.asarray(b_attn, dtype=np.float32)
    W_proj = np.asarray(W_proj, dtype=np.float32)
    b_proj = np.asarray(b_proj, dtype=np.float32)

    qk_bias = bool(np.any(b_attn[:2 * C]))
    v_bias = bool(np.any(b_attn[2 * C:]))

    key = (qk_bias, v_bias)
    if key not in _BUILD_CACHE:
        _BUILD_CACHE[key] = _build_nc(qk_bias, v_bias)
    nc = _BUILD_CACHE[key]

    mstrip = _band_mask_strip()
    in_maps = []
    for c in range(8):
        b, g = c // 2, c % 2
        fsl = slice(FG * g, FG * (g + 1))
        # partition-major host layouts so every device load is one big
        # contiguous DMA: xt [p][t,k,f], wqk [p][half,k,f], wv [p][k,f],
        # wp [p][k,c]
        xt_h = (x[b].T.reshape(KC, P, NT, P).transpose(1, 2, 0, 3)
                .reshape(P, NT * KC * P))
        # wqk laid out per q/k feature tile: [p][ft 8][k 8][128]
        wqk_h = (np.concatenate(
            [W_attn[:, fsl], W_attn[:, C + FG * g:C + FG * (g + 1)]], axis=1)
            .reshape(KC, P, 8, P).transpose(1, 2, 0, 3)
            .reshape(P, 2 * KC * FG))
        wv_h = (W_attn[:, 2 * C + FG * g:2 * C + FG * (g + 1)]
                .reshape(KC, P, FG).transpose(1, 0, 2).reshape(P, KC * FG))
        wp_h = (W_proj[fsl, :].reshape(FG // P, P, C).transpose(1, 0, 2)
                .reshape(P, (FG // P) * C))
        im = {
            "xt": np.ascontiguousarray(xt_h).astype(ml_dtypes.bfloat16),
            "wqk": np.ascontiguousarray(wqk_h).astype(ml_dtypes.bfloat16),
            "wv": np.ascontiguousarray(wv_h).astype(ml_dtypes.bfloat16),
            "wp": np.ascontiguousarray(wp_h).astype(ml_dtypes.bfloat16),
            "mstrip": mstrip,
        }
        if qk_bias:
            bq = b_attn[fsl]
            bk = b_attn[C + FG * g:C + FG * (g + 1)]
            im["bqk"] = np.concatenate([bq, bk]).reshape(8, P).astype(np.float32)
        if v_bias:
            bv = b_attn[2 * C + FG * g:2 * C + FG * (g + 1)]
            im["bv"] = np.broadcast_to(
                bv.reshape(1, HG * D), (P, HG * D)
            ).astype(ml_dtypes.bfloat16)
        in_maps.append(im)

    res = run_bass_kernel_spmd(nc, in_maps, list(range(8)))
    LAST_RESULTS = res

    out = np.empty((B, T, C), dtype=np.float32)
    for b in range(B):
        out[b] = (res.results[2 * b]["y"].astype(np.float32)
                  + res.results[2 * b + 1]["y"].astype(np.float32)
                  + b_proj)
    return out
